# revision 2
# baseline (speedup 1.0000x reference)
"""Trainium2 Bass kernel for nn_CarNet (scatter_memory).

Math (per batch b):
    arg1[f, r]  = sum_l w[l] * x[l, f, r]          (L=64 weighted reduction)
    out[f, t]   = sum_r arg1[f, r] * cw[t, r]      (role remap via car_weight)
    entropy[b]  = -(sum_l p log p) / log(L),  p = w / sum(w)
    max_w[b]    = max_l w[l]

Sharding: data-parallel over batch B=8 across the 8 NeuronCores (one batch
per core); car_weight is replicated (pre-transposed on host so its
contraction dim lands on SBUF partitions).

Per-core implementation:
  - x[b] viewed as (4096, 1024) is streamed in 8 DMA chunks of (128, 4, 1024).
    Row 512*i + 4*p + c of the chunk lands on partition p, slab c -> 16KB
    contiguous per partition per DMA (good descriptor shape).
  - Stage 1 runs on the TensorEngine: for each (chunk i, slab c) a host-built
    stationary matrix W2[p, f] = w[l(row)] * delta(f, row%64) folds the
    l-weighted sum into a 64-partition PSUM accumulator (64 matmuls,
    PSUM-accumulated).  This keeps the reduction off the (slow for fp32)
    VectorEngine and overlaps fully with the DMA stream.
  - arg1 is transposed 128 columns at a time via PE transpose, then stage 2
    is 16 accumulating matmuls against the replicated car_weight^T.
  - entropy/max_w are computed on-chip from the (1, 64) weight row.
"""

import functools

import numpy as np

B, L, F, R, T = 8, 64, 64, 1024, 1024
NCORES = 8
ROWS = L * F  # 4096
N_CHUNKS = 8
ROWS_PER_CHUNK = ROWS // N_CHUNKS  # 512
SLABS_PER_CHUNK = ROWS_PER_CHUNK // 128  # 4
N_SLABS = N_CHUNKS * SLABS_PER_CHUNK  # 32

# "f32" (exact) or "bf16" (halves DMA traffic; matmuls accumulate in fp32)
COMPUTE = "f32"


def _np_compute_dtype():
    if COMPUTE == "bf16":
        import ml_dtypes

        return ml_dtypes.bfloat16
    return np.float32


@functools.lru_cache(maxsize=1)
def _build():
    import concourse.bacc as bacc
    import concourse.mybir as mybir
    import concourse.tile as tile
    from concourse.masks import make_identity

    fp32 = mybir.dt.float32
    cdt = mybir.dt.bfloat16 if COMPUTE == "bf16" else fp32

    nc = bacc.Bacc("TRN2", target_bir_lowering=False, debug=False,
                   num_devices=NCORES)

    xb = nc.dram_tensor("xb", [ROWS, R], cdt, kind="ExternalInput")
    w2 = nc.dram_tensor("w2", [128, N_SLABS, F], cdt, kind="ExternalInput")
    wrow = nc.dram_tensor("wrow", [1, L], fp32, kind="ExternalInput")
    cwt = nc.dram_tensor("cwt", [R, T], cdt, kind="ExternalInput")
    out = nc.dram_tensor("out", [F, T], fp32, kind="ExternalOutput")
    stats = nc.dram_tensor("stats", [1, 2], fp32, kind="ExternalOutput")

    with tile.TileContext(nc) as tc:
        with (
            tc.tile_pool(name="singles", bufs=1) as singles,
            tc.tile_pool(name="xpool", bufs=3) as xpool,
            tc.tile_pool(name="tail", bufs=3) as tail,
            tc.tile_pool(name="ps_acc", bufs=1, space="PSUM") as ps_acc,
            tc.tile_pool(name="ps_small", bufs=2, space="PSUM") as ps_small,
        ):
            # ---- constants / small inputs ----
            w2_sb = singles.tile([128, N_SLABS, F], cdt)
            nc.sync.dma_start(out=w2_sb[:], in_=w2.ap())
            wrow_sb = singles.tile([1, L], fp32)
            nc.sync.dma_start(out=wrow_sb[:], in_=wrow.ap())
            ident = singles.tile([128, 128], fp32)
            make_identity(nc, ident[:])

            # ---- stage 1: l-weighted reduction, PSUM-accumulated on PE ----
            psum_arg1 = ps_acc.tile([F, R], fp32)
            for i in range(N_CHUNKS):
                xt = xpool.tile([128, SLABS_PER_CHUNK, R], cdt)
                src = xb.ap()[i * ROWS_PER_CHUNK:(i + 1) * ROWS_PER_CHUNK, :]
                nc.sync.dma_start(
                    out=xt[:], in_=src.rearrange("(p c) n -> p c n",
                                                 c=SLABS_PER_CHUNK))
                for c in range(SLABS_PER_CHUNK):
                    s = i * SLABS_PER_CHUNK + c
                    for n in range(R // 512):
                        nc.tensor.matmul(
                            out=psum_arg1[:, n * 512:(n + 1) * 512],
                            lhsT=w2_sb[:, s, :],
                            rhs=xt[:, c, n * 512:(n + 1) * 512],
                            start=(s == 0),
                            stop=(s == N_SLABS - 1),
                        )

            # ---- replicated car_weight^T, loaded behind the x stream ----
            cwt_sb = singles.tile([128, R // 128, T], cdt)
            for q in range(4):
                src = cwt.ap()[q * 256:(q + 1) * 256, :]
                nc.sync.dma_start(
                    out=cwt_sb[:, 2 * q:2 * q + 2, :],
                    in_=src.rearrange("(c p) n -> p c n", p=128))

            # ---- transpose arg1 -> (r, f) chunks ----
            arg1t_sb = singles.tile([128, R // 128, F], cdt)
            for c in range(R // 128):
                a1c = tail.tile([F, 128], fp32)
                nc.vector.tensor_copy(out=a1c[:],
                                      in_=psum_arg1[:, c * 128:(c + 1) * 128])
                pt = ps_small.tile([128, F], fp32)
                nc.tensor.transpose(out=pt[:], in_=a1c[:], identity=ident[:F, :F])
                nc.vector.tensor_copy(out=arg1t_sb[:, c, :], in_=pt[:])

            # ---- stage 2: out = arg1 @ cw^T ----
            psum_out = ps_acc.tile([F, T], fp32)
            for c in range(R // 128):
                for n in range(T // 512):
                    nc.tensor.matmul(
                        out=psum_out[:, n * 512:(n + 1) * 512],
                        lhsT=arg1t_sb[:, c, :],
                        rhs=cwt_sb[:, c, n * 512:(n + 1) * 512],
                        start=(c == 0),
                        stop=(c == R // 128 - 1),
                    )
            out_sb = singles.tile([F, T], fp32)
            nc.scalar.copy(out=out_sb[:], in_=psum_out[:])
            nc.sync.dma_start(out=out.ap(), in_=out_sb[:])

            # ---- entropy + max over the (1, 64) weight row ----
            mybir_ = mybir
            st = singles.tile([1, 16], fp32)  # scratch scalars on partition 0
            lw = singles.tile([1, L], fp32)
            stats_sb = singles.tile([1, 2], fp32)
            s_sum = st[:, 0:1]
            s_max = st[:, 1:2]
            s_swlw = st[:, 2:3]
            s_inv = st[:, 3:4]
            s_ls = st[:, 4:5]
            s_t1 = st[:, 5:6]
            s_t2 = st[:, 6:7]
            nc.vector.reduce_sum(out=s_sum, in_=wrow_sb[:],
                                 axis=mybir_.AxisListType.X)
            nc.vector.reduce_max(out=s_max, in_=wrow_sb[:],
                                 axis=mybir_.AxisListType.X)
            nc.scalar.activation(out=lw[:], in_=wrow_sb[:],
                                 func=mybir_.ActivationFunctionType.Ln)
            # s_swlw = sum(w * ln w) via fused accumulate
            dummy = singles.tile([1, L], fp32)
            nc.vector.scalar_tensor_tensor(
                out=dummy[:], in0=wrow_sb[:], scalar=1.0, in1=lw[:],
                op0=mybir_.AluOpType.mult, op1=mybir_.AluOpType.mult,
                accum_out=s_swlw)
            nc.vector.reciprocal(out=s_inv, in_=s_sum)
            nc.scalar.activation(out=s_ls, in_=s_sum,
                                 func=mybir_.ActivationFunctionType.Ln)
            nc.vector.tensor_tensor(out=s_t1, in0=s_swlw, in1=s_inv,
                                    op=mybir_.AluOpType.mult)
            nc.vector.tensor_tensor(out=s_t2, in0=s_ls, in1=s_t1,
                                    op=mybir_.AluOpType.subtract)
            nc.vector.tensor_scalar_mul(stats_sb[:, 0:1], s_t2,
                                        float(1.0 / np.log(L)))
            nc.vector.tensor_copy(out=stats_sb[:, 1:2], in_=s_max)
            nc.sync.dma_start(out=stats.ap(), in_=stats_sb[:])

    nc.finalize()
    return nc


def _build_w2(w: np.ndarray) -> np.ndarray:
    """Stationary stage-1 weights: W2h[p, s, f] = w[row//64] * (row%64 == f)
    with row = 512*(s//4) + 4*p + (s%4)."""
    p = np.arange(128)[:, None]
    s = np.arange(N_SLABS)[None, :]
    row = 512 * (s // SLABS_PER_CHUNK) + SLABS_PER_CHUNK * p + (s % SLABS_PER_CHUNK)
    w2 = np.zeros((128, N_SLABS, F), dtype=np.float32)
    pp, ss = np.meshgrid(np.arange(128), np.arange(N_SLABS), indexing="ij")
    w2[pp, ss, row % 64] = w[row // 64]
    return w2


def _run(x, arg1_weight, car_weight, trace=False):
    from concourse.bass_utils import run_bass_kernel_spmd

    nc = _build()
    cd = _np_compute_dtype()
    x = np.asarray(x, dtype=np.float32)
    arg1_weight = np.asarray(arg1_weight, dtype=np.float32)
    car_weight = np.asarray(car_weight, dtype=np.float32)
    cwt = np.ascontiguousarray(car_weight.T).astype(cd)
    in_maps = []
    for b in range(B):
        in_maps.append({
            "xb": np.ascontiguousarray(x[b].reshape(ROWS, R)).astype(cd),
            "w2": _build_w2(arg1_weight[b]).astype(cd),
            "wrow": np.ascontiguousarray(arg1_weight[b:b + 1]),
            "cwt": cwt,
        })
    res = run_bass_kernel_spmd(nc, in_maps, core_ids=list(range(NCORES)),
                               trace=trace)
    outs = res.results
    output = np.stack([r["out"] for r in outs]).astype(np.float32)
    entropy = np.array([r["stats"][0, 0] for r in outs], dtype=np.float32)
    max_w = np.array([r["stats"][0, 1] for r in outs], dtype=np.float32)
    return (output, entropy, max_w), res


def kernel(x, arg1_weight, car_weight):
    (output, entropy, max_w), _ = _run(x, arg1_weight, car_weight)
    return output, entropy, max_w


# revision 3
# speedup vs baseline: 1.9307x; 1.9307x over previous
"""Trainium2 Bass kernel for nn_CarNet (scatter_memory).

Math (per batch b):
    arg1[f, r]  = sum_l w[l] * x[l, f, r]          (L=64 weighted reduction)
    out[f, t]   = sum_r arg1[f, r] * cw[t, r]      (role remap via car_weight)
    entropy[b]  = -(sum_l p log p) / log(L),  p = w / sum(w)
    max_w[b]    = max_l w[l]

Sharding: data-parallel over batch B=8 across the 8 NeuronCores (one batch
per core); car_weight is replicated (pre-transposed on host so its
contraction dim lands on SBUF partitions).

Per-core implementation:
  - x[b] viewed as (4096, 1024) is streamed in 8 DMA chunks of (128, 4, 1024).
    Row 512*i + 4*p + c of the chunk lands on partition p, slab c -> 16KB
    contiguous per partition per DMA (good descriptor shape).
  - Stage 1 runs on the TensorEngine: for each (chunk i, slab c) a host-built
    stationary matrix W2[p, f] = w[l(row)] * delta(f, row%64) folds the
    l-weighted sum into a 64-partition PSUM accumulator (64 matmuls,
    PSUM-accumulated).  This keeps the reduction off the (slow for fp32)
    VectorEngine and overlaps fully with the DMA stream.
  - arg1 is transposed 128 columns at a time via PE transpose, then stage 2
    is 16 accumulating matmuls against the replicated car_weight^T.
  - entropy/max_w are computed on-chip from the (1, 64) weight row.
"""

import functools

import numpy as np

B, L, F, R, T = 8, 64, 64, 1024, 1024
NCORES = 8
ROWS = L * F  # 4096
N_CHUNKS = 8
ROWS_PER_CHUNK = ROWS // N_CHUNKS  # 512
SLABS_PER_CHUNK = ROWS_PER_CHUNK // 128  # 4
N_SLABS = N_CHUNKS * SLABS_PER_CHUNK  # 32

# "f32" (exact) or "bf16" (halves DMA traffic; matmuls accumulate in fp32)
COMPUTE = "bf16"


def _np_compute_dtype():
    if COMPUTE == "bf16":
        import ml_dtypes

        return ml_dtypes.bfloat16
    return np.float32


@functools.lru_cache(maxsize=1)
def _build():
    import concourse.bacc as bacc
    import concourse.mybir as mybir
    import concourse.tile as tile
    from concourse.masks import make_identity

    fp32 = mybir.dt.float32
    cdt = mybir.dt.bfloat16 if COMPUTE == "bf16" else fp32

    nc = bacc.Bacc("TRN2", target_bir_lowering=False, debug=False,
                   num_devices=NCORES)

    xb = nc.dram_tensor("xb", [ROWS, R], cdt, kind="ExternalInput")
    w2 = nc.dram_tensor("w2", [128, N_SLABS, F], cdt, kind="ExternalInput")
    wrow = nc.dram_tensor("wrow", [1, L], fp32, kind="ExternalInput")
    cwt = nc.dram_tensor("cwt", [R, T], cdt, kind="ExternalInput")
    out = nc.dram_tensor("out", [F, T], fp32, kind="ExternalOutput")
    stats = nc.dram_tensor("stats", [1, 2], fp32, kind="ExternalOutput")

    with tile.TileContext(nc) as tc:
        with (
            tc.tile_pool(name="singles", bufs=1) as singles,
            tc.tile_pool(name="xpool", bufs=3) as xpool,
            tc.tile_pool(name="tail", bufs=3) as tail,
            tc.tile_pool(name="ps_acc", bufs=1, space="PSUM") as ps_acc,
            tc.tile_pool(name="ps_small", bufs=2, space="PSUM") as ps_small,
        ):
            # ---- constants / small inputs ----
            w2_sb = singles.tile([128, N_SLABS, F], cdt)
            nc.sync.dma_start(out=w2_sb[:], in_=w2.ap())
            wrow_sb = singles.tile([1, L], fp32)
            nc.sync.dma_start(out=wrow_sb[:], in_=wrow.ap())
            ident = singles.tile([128, 128], fp32)
            make_identity(nc, ident[:])

            # ---- stage 1: l-weighted reduction, PSUM-accumulated on PE ----
            psum_arg1 = ps_acc.tile([F, R], fp32)
            for i in range(N_CHUNKS):
                xt = xpool.tile([128, SLABS_PER_CHUNK, R], cdt)
                src = xb.ap()[i * ROWS_PER_CHUNK:(i + 1) * ROWS_PER_CHUNK, :]
                nc.sync.dma_start(
                    out=xt[:], in_=src.rearrange("(p c) n -> p c n",
                                                 c=SLABS_PER_CHUNK))
                for c in range(SLABS_PER_CHUNK):
                    s = i * SLABS_PER_CHUNK + c
                    for n in range(R // 512):
                        nc.tensor.matmul(
                            out=psum_arg1[:, n * 512:(n + 1) * 512],
                            lhsT=w2_sb[:, s, :],
                            rhs=xt[:, c, n * 512:(n + 1) * 512],
                            start=(s == 0),
                            stop=(s == N_SLABS - 1),
                        )

            # ---- replicated car_weight^T, loaded behind the x stream ----
            cwt_sb = singles.tile([128, R // 128, T], cdt)
            for q in range(4):
                src = cwt.ap()[q * 256:(q + 1) * 256, :]
                nc.sync.dma_start(
                    out=cwt_sb[:, 2 * q:2 * q + 2, :],
                    in_=src.rearrange("(c p) n -> p c n", p=128))

            # ---- transpose arg1 -> (r, f) chunks ----
            arg1t_sb = singles.tile([128, R // 128, F], cdt)
            for c in range(R // 128):
                a1c = tail.tile([F, 128], fp32)
                nc.vector.tensor_copy(out=a1c[:],
                                      in_=psum_arg1[:, c * 128:(c + 1) * 128])
                pt = ps_small.tile([128, F], fp32)
                nc.tensor.transpose(out=pt[:], in_=a1c[:], identity=ident[:F, :F])
                nc.vector.tensor_copy(out=arg1t_sb[:, c, :], in_=pt[:])

            # ---- stage 2: out = arg1 @ cw^T ----
            psum_out = ps_acc.tile([F, T], fp32)
            for c in range(R // 128):
                for n in range(T // 512):
                    nc.tensor.matmul(
                        out=psum_out[:, n * 512:(n + 1) * 512],
                        lhsT=arg1t_sb[:, c, :],
                        rhs=cwt_sb[:, c, n * 512:(n + 1) * 512],
                        start=(c == 0),
                        stop=(c == R // 128 - 1),
                    )
            out_sb = singles.tile([F, T], fp32)
            nc.scalar.copy(out=out_sb[:], in_=psum_out[:])
            nc.sync.dma_start(out=out.ap(), in_=out_sb[:])

            # ---- entropy + max over the (1, 64) weight row ----
            mybir_ = mybir
            st = singles.tile([1, 16], fp32)  # scratch scalars on partition 0
            lw = singles.tile([1, L], fp32)
            stats_sb = singles.tile([1, 2], fp32)
            s_sum = st[:, 0:1]
            s_max = st[:, 1:2]
            s_swlw = st[:, 2:3]
            s_inv = st[:, 3:4]
            s_ls = st[:, 4:5]
            s_t1 = st[:, 5:6]
            s_t2 = st[:, 6:7]
            nc.vector.reduce_sum(out=s_sum, in_=wrow_sb[:],
                                 axis=mybir_.AxisListType.X)
            nc.vector.reduce_max(out=s_max, in_=wrow_sb[:],
                                 axis=mybir_.AxisListType.X)
            nc.scalar.activation(out=lw[:], in_=wrow_sb[:],
                                 func=mybir_.ActivationFunctionType.Ln)
            # s_swlw = sum(w * ln w) via fused accumulate
            dummy = singles.tile([1, L], fp32)
            nc.vector.scalar_tensor_tensor(
                out=dummy[:], in0=wrow_sb[:], scalar=1.0, in1=lw[:],
                op0=mybir_.AluOpType.mult, op1=mybir_.AluOpType.mult,
                accum_out=s_swlw)
            nc.vector.reciprocal(out=s_inv, in_=s_sum)
            nc.scalar.activation(out=s_ls, in_=s_sum,
                                 func=mybir_.ActivationFunctionType.Ln)
            nc.vector.tensor_tensor(out=s_t1, in0=s_swlw, in1=s_inv,
                                    op=mybir_.AluOpType.mult)
            nc.vector.tensor_tensor(out=s_t2, in0=s_ls, in1=s_t1,
                                    op=mybir_.AluOpType.subtract)
            nc.vector.tensor_scalar_mul(stats_sb[:, 0:1], s_t2,
                                        float(1.0 / np.log(L)))
            nc.vector.tensor_copy(out=stats_sb[:, 1:2], in_=s_max)
            nc.sync.dma_start(out=stats.ap(), in_=stats_sb[:])

    nc.finalize()
    return nc


def _build_w2(w: np.ndarray) -> np.ndarray:
    """Stationary stage-1 weights: W2h[p, s, f] = w[row//64] * (row%64 == f)
    with row = 512*(s//4) + 4*p + (s%4)."""
    p = np.arange(128)[:, None]
    s = np.arange(N_SLABS)[None, :]
    row = 512 * (s // SLABS_PER_CHUNK) + SLABS_PER_CHUNK * p + (s % SLABS_PER_CHUNK)
    w2 = np.zeros((128, N_SLABS, F), dtype=np.float32)
    pp, ss = np.meshgrid(np.arange(128), np.arange(N_SLABS), indexing="ij")
    w2[pp, ss, row % 64] = w[row // 64]
    return w2


def _run(x, arg1_weight, car_weight, trace=False):
    from concourse.bass_utils import run_bass_kernel_spmd

    nc = _build()
    cd = _np_compute_dtype()
    x = np.asarray(x, dtype=np.float32)
    arg1_weight = np.asarray(arg1_weight, dtype=np.float32)
    car_weight = np.asarray(car_weight, dtype=np.float32)
    cwt = np.ascontiguousarray(car_weight.T).astype(cd)
    in_maps = []
    for b in range(B):
        in_maps.append({
            "xb": np.ascontiguousarray(x[b].reshape(ROWS, R)).astype(cd),
            "w2": _build_w2(arg1_weight[b]).astype(cd),
            "wrow": np.ascontiguousarray(arg1_weight[b:b + 1]),
            "cwt": cwt,
        })
    res = run_bass_kernel_spmd(nc, in_maps, core_ids=list(range(NCORES)),
                               trace=trace)
    outs = res.results
    output = np.stack([r["out"] for r in outs]).astype(np.float32)
    entropy = np.array([r["stats"][0, 0] for r in outs], dtype=np.float32)
    max_w = np.array([r["stats"][0, 1] for r in outs], dtype=np.float32)
    return (output, entropy, max_w), res


def kernel(x, arg1_weight, car_weight):
    (output, entropy, max_w), _ = _run(x, arg1_weight, car_weight)
    return output, entropy, max_w
